# revision 1
# baseline (speedup 1.0000x reference)
"""ADMM solver block (nn_ADMMSolverBlock) — Trainium2 Bass kernel, 8 NeuronCores.

Strategy
--------
The reference ADMM iteration acts independently on each of the 128
(channel, batch) columns of the [N=4096, B] problem, so the batch/column
dimension is sharded 16-columns-per-core across the 8 cores (data parallel,
no collectives).  The (I + rho*D^T D)^{-1} solve is separable on the 64x64
grid: D = [I (x) Ax ; Ay (x) I] (verified at runtime by random probes), so
the solve is two 64-point eigenbasis transforms (eigendecompositions of the
1D path Laplacians Ax^T Ax and Ay^T Ay, computed on host from the passed D —
this mirrors the torch module, which precomputes B^{-1} at init) plus a
per-frequency scale S = 1/(1 + rho*(mu_k + nu_l)).

Per-core device layout: plane [128, 512] f32; partition p = c*64 + i
(c = channel, i = grid row in flipped-H space); free f = lb*64 + j
(lb = local batch 0..7, j = grid col).  Two independent column streams
(lb 0..3 / 4..7) are interleaved so the engines overlap.

Device iteration (stream-local):
  Xh = Yh_prev + DCT(rho * D^T W),  W folded as C - 1.1*clip - 0.9*beta_old
       (the forward DCT is distributed over those three terms as extra
        accumulating matmuls, so the W state never materializes)
  Yh = S .* Xh          (spectral state, carries DCT(Fc) across iterations)
  Q  = IDCT(Yh);  DQy via a fused (Ay@Vi)^T matmul from the same operand
  C  = D Q + beta;  clip = clamp(C, +-th)
  beta' = 0.9*beta + 0.1*clip   (identity-matmul accumulation in PSUM)
All matmuls run as float32r (1 cycle/row at free>=256; measured end-to-end
relative error ~6e-4).
"""

import numpy as np

import concourse.bacc as bacc
import concourse.mybir as mybir
import concourse.tile as tile
from concourse.bass_utils import run_bass_kernel_spmd

F32 = mybir.dt.float32
F32R = mybir.dt.float32r
ALU = mybir.AluOpType

RHO, LAMB, ETA, T = 0.1, 0.01, 0.1, 4
TH = LAMB / RHO
HH = 64
WW = 64
N = HH * WW
BATCH = 64
NCORES = 8
NLB = BATCH // NCORES

CONST_NAMES = [
    "c_vi", "ident", "c_vj", "s_t", "c_vjt", "c_vit", "c_ayvit",
    "c_rayvi", "c_vim11", "c_vim09", "c_rayvim11", "c_rayvim09",
    "c_raxvj", "c_i", "c_i09", "c_i01",
]
CONST_SPLITS = [3, 7]  # const-block DMA split points (by tensor index)


def _bd(m):
    out = np.zeros((128, 128), np.float32)
    out[:64, :64] = m
    out[64:, 64:] = m
    return out


def host_constants(D):
    D = np.asarray(D)
    Ax = D[0][:WW, :WW].astype(np.float64)
    Ay = D[1][::WW, ::WW].astype(np.float64)

    rng = np.random.default_rng(0)
    for _ in range(2):
        v = rng.standard_normal(N).astype(np.float32)
        vg = v.reshape(HH, WW)
        if not np.allclose(D[0] @ v, (vg @ Ax.T.astype(np.float32)).ravel(), atol=1e-3):
            raise ValueError("D[0] does not have the expected I (x) Ax structure")
        if not np.allclose(D[1] @ v, (Ay.astype(np.float32) @ vg).ravel(), atol=1e-3):
            raise ValueError("D[1] does not have the expected Ay (x) I structure")

    nu, Vj = np.linalg.eigh(Ax.T @ Ax)
    mu, Vi = np.linalg.eigh(Ay.T @ Ay)
    S = 1.0 / (1.0 + RHO * (mu[:, None] + nu[None, :]))  # S[k, l]

    Vi32 = Vi.astype(np.float32)
    Vj32 = Vj.astype(np.float32)
    rayvi = RHO * (Ay @ Vi)

    # s_t[(lbr,l), (pair,(c,k))] = S[k, l]  (transposed spectral layout)
    s_t = np.zeros((128, 256), np.float32)
    Sf = S.astype(np.float32)
    for lbr in range(2):
        for pair in range(2):
            for c in range(2):
                s_t[lbr * 64:(lbr + 1) * 64,
                    pair * 128 + c * 64: pair * 128 + (c + 1) * 64] = Sf.T

    eye = np.eye(128, dtype=np.float32)
    return {
        "c_vi": _bd(Vi32),
        "ident": eye,
        "c_vj": _bd(Vj32),
        "s_t": s_t,
        "c_vjt": _bd(Vj32.T),
        "c_vit": _bd(Vi32.T),
        "c_ayvit": _bd((Ay @ Vi).T.astype(np.float32)),
        "c_rayvi": _bd(rayvi.astype(np.float32)),
        "c_vim11": _bd((-(1.0 + ETA) * Vi).astype(np.float32)),
        "c_vim09": _bd((-(1.0 - ETA) * Vi).astype(np.float32)),
        "c_rayvim11": _bd((-(1.0 + ETA) * rayvi).astype(np.float32)),
        "c_rayvim09": _bd((-(1.0 - ETA) * rayvi).astype(np.float32)),
        "c_raxvj": _bd((RHO * (Ax @ Vj)).astype(np.float32)),
        "c_i": eye,
        "c_i09": (1.0 - ETA) * eye,
        "c_i01": ETA * eye,
    }


def pack_consts(consts):
    widths = [(256 if n == "s_t" else 128) for n in CONST_NAMES]
    blk = np.zeros((128, sum(widths)), np.float32)
    off = 0
    for n, w in zip(CONST_NAMES, widths):
        blk[:, off:off + w] = consts[n]
        off += w
    return blk


def host_pack(F):
    Fg = np.flip(np.asarray(F), axis=2).transpose(1, 0, 2, 3)  # [c, b, i, j]
    per_core = []
    for r in range(NCORES):
        blk = Fg[:, NLB * r:NLB * (r + 1)]
        per_core.append(np.ascontiguousarray(
            blk.transpose(0, 2, 1, 3).reshape(128, 512)))
    return per_core


def host_unpack(outs):
    Q = np.zeros((BATCH, 2, HH, WW), np.float32)
    for r, o in enumerate(outs):
        blk = o.reshape(2, HH, NLB, WW).transpose(0, 2, 1, 3)  # [c, lb, i, j]
        Q[NLB * r:NLB * (r + 1)] = blk.transpose(1, 0, 2, 3)
    return np.flip(Q, axis=2).copy()


def _r(ap):
    return ap.bitcast(F32R)


def _mm(nc, out, lhsT, rhs, start=True, stop=True):
    nc.tensor.matmul(out, lhsT=lhsT, rhs=rhs, start=start, stop=stop)


def _tr(nc, out, in_, ident):
    nc.tensor.matmul(_r(out), lhsT=in_, rhs=ident, is_transpose=True,
                     start=True, stop=True)


def build(reps=1, debug=False):
    nc = bacc.Bacc(
        "TRN2",
        target_bir_lowering=False,
        debug=debug,
        enable_asserts=True,
        num_devices=NCORES,
    )
    d_in = nc.dram_tensor("x0", [128, 512], F32R, kind="ExternalInput")
    d_out = nc.dram_tensor("out", [128, 512], F32, kind="ExternalOutput")
    widths = [(256 if n == "s_t" else 128) for n in CONST_NAMES]
    total_w = sum(widths)
    d_cb = nc.dram_tensor("consts", [128, total_w], F32R, kind="ExternalInput")

    with tile.TileContext(nc) as tc:
        with tc.tile_pool(name="const", bufs=1) as cpool, \
             tc.tile_pool(name="state", bufs=1) as spool, \
             tc.tile_pool(name="work", bufs=2) as wpool, \
             tc.tile_pool(name="psum", bufs=1, space="PSUM") as pspool:

            cblk = cpool.tile([128, total_w], F32R, tag="cblk", name="cblk")
            bounds = [0] + [sum(widths[:k]) for k in CONST_SPLITS] + [total_w]
            for bi in range(len(bounds) - 1):
                lo, hi = bounds[bi], bounds[bi + 1]
                nc.sync.dma_start(out=cblk[:, lo:hi], in_=d_cb[:, lo:hi])
            ct = {}
            off = 0
            for name, w in zip(CONST_NAMES, widths):
                ct[name] = cblk[:, off:off + w]
                off += w

            for _ in range(reps):
                _rep(nc, spool, wpool, pspool, ct, d_in, d_out)

    nc.compile()
    return nc


def _rep(nc, spool, wpool, pspool, ct, d_in, d_out):
    Fc = spool.tile([128, 512], F32R, tag="Fc", name="Fc")
    nc.gpsimd.dma_start(out=Fc[:], in_=d_in[:])

    st = []
    for s in range(2):
        S = {
            "B": spool.tile([128, 512], F32R, tag=f"B{s}", name=f"B{s}"),
            "Yh": spool.tile([128, 256], F32R, tag=f"Yh{s}", name=f"Yh{s}"),
            "tx": spool.tile([128, 256], F32, tag=f"tx{s}", name=f"tx{s}"),
            "Ct": spool.tile([128, 512], F32R, tag=f"Ct{s}", name=f"Ct{s}"),
            "Clip": spool.tile([128, 512], F32R, tag=f"Cl{s}", name=f"Cl{s}"),
            "s": s,
            "fs": slice(256 * s, 256 * s + 256),
            "psb": None,
        }
        t3 = S["tx"][:].rearrange("p (c j) -> p c j", j=64)
        nc.gpsimd.memset(t3[:, :, 63:64], 0.0)
        st.append(S)

    for t in range(T):
        last = t == T - 1
        for s in range(2):
            _fwd(nc, wpool, pspool, ct, st[s], Fc, t)
        for s in range(2):
            _inv(nc, wpool, pspool, ct, st[s], d_out, last)
        if last:
            break
        for s in range(2):
            _post(nc, wpool, pspool, ct, st[s], t == 0)


def _fwd(nc, wpool, pspool, ct, S, Fc, t):
    s, Yh = S["s"], S["Yh"]
    if t == 0:
        psA = pspool.tile([128, 256], F32, tag=f"pa{s}", name=f"psA{s}")
        _mm(nc, psA[:], ct["c_vi"], Fc[:, S["fs"]])
        Asb = wpool.tile([128, 256], F32R, tag=f"asb{s}", name=f"asb{s}")
        nc.scalar.copy(Asb[:], psA[:])
        psT = pspool.tile([128, 256], F32, tag=f"pb{s}", name=f"psT{s}")
        for p in range(2):
            sl = slice(128 * p, 128 * (p + 1))
            _tr(nc, psT[:, sl], Asb[:, sl], ct["ident"])
        Tsb = wpool.tile([128, 256], F32R, tag=f"tsb{s}", name=f"tsb{s}")
        nc.scalar.copy(Tsb[:], psT[:])
        psB = pspool.tile([128, 256], F32, tag=f"pc{s}", name=f"psB{s}")
        _mm(nc, psB[:], ct["c_vj"], Tsb[:])
    else:
        Ct, Clip, Bt = S["Ct"], S["Clip"], S["B"]
        psA = pspool.tile([128, 512], F32, tag=f"pa{s}", name=f"psA{s}")
        # x half: bd(Vi) @ (Cx - 1.1 clipx - 0.9 bx_old)
        _mm(nc, psA[:, 0:256], ct["c_vi"], Ct[:, 0:256], start=True, stop=False)
        _mm(nc, psA[:, 0:256], ct["c_vim11"], Clip[:, 0:256],
            start=False, stop=(t == 1))
        if t > 1:
            _mm(nc, psA[:, 0:256], ct["c_vim09"], Bt[:, 0:256],
                start=False, stop=True)
        # y half with bd(rho*Ay@Vi) variants
        _mm(nc, psA[:, 256:512], ct["c_rayvi"], Ct[:, 256:512],
            start=True, stop=False)
        _mm(nc, psA[:, 256:512], ct["c_rayvim11"], Clip[:, 256:512],
            start=False, stop=(t == 1))
        if t > 1:
            _mm(nc, psA[:, 256:512], ct["c_rayvim09"], Bt[:, 256:512],
                start=False, stop=True)
        # beta_old fully consumed: land the deferred beta update
        if S["psb"] is not None:
            if t < T - 1:
                nc.vector.tensor_copy(Bt[:], S["psb"][:])
            S["psb"] = None
        Asb = wpool.tile([128, 512], F32R, tag=f"asb{s}", name=f"asb{s}")
        nc.scalar.copy(Asb[:], psA[:])
        psT = pspool.tile([128, 512], F32, tag=f"pb{s}", name=f"psT{s}")
        for p in range(4):
            sl = slice(128 * p, 128 * (p + 1))
            _tr(nc, psT[:, sl], Asb[:, sl], ct["ident"])
        Tsb = wpool.tile([128, 512], F32R, tag=f"tsb{s}", name=f"tsb{s}")
        nc.scalar.copy(Tsb[:], psT[:])
        psB = pspool.tile([128, 256], F32, tag=f"pc{s}", name=f"psB{s}")
        _mm(nc, psB[:], ct["c_raxvj"], Tsb[:, 0:256], start=True, stop=False)
        _mm(nc, psB[:], ct["c_vj"], Tsb[:, 256:512], start=False, stop=False)
        _mm(nc, psB[:], ct["c_i"], Yh[:], start=False, stop=True)

    nc.vector.tensor_tensor(Yh[:], psB[:], ct["s_t"], ALU.mult)


def _inv(nc, wpool, pspool, ct, S, d_out, last):
    s, Yh = S["s"], S["Yh"]
    psE = pspool.tile([128, 256], F32, tag=f"pa{s}", name=f"psE{s}")
    _mm(nc, psE[:], ct["c_vjt"], Yh[:])
    Esb = wpool.tile([128, 256], F32R, tag=f"esb{s}", name=f"esb{s}")
    nc.vector.tensor_copy(Esb[:], psE[:])
    psF = pspool.tile([128, 256], F32, tag=f"pb{s}", name=f"psF{s}")
    for p in range(2):
        sl = slice(128 * p, 128 * (p + 1))
        _tr(nc, psF[:, sl], Esb[:, sl], ct["ident"])
    Fsb = wpool.tile([128, 256], F32R, tag=f"fsb{s}", name=f"fsb{s}")
    nc.scalar.copy(Fsb[:], psF[:])
    psQ = pspool.tile([128, 256], F32, tag=f"pc{s}", name=f"psQ{s}")
    _mm(nc, psQ[:], ct["c_vit"], Fsb[:])
    Qsb = wpool.tile([128, 256], F32, tag=f"qsb{s}", name=f"qsb{s}")
    nc.vector.tensor_copy(Qsb[:], psQ[:])
    if last:
        nc.sync.dma_start(out=d_out[:, S["fs"]], in_=Qsb[:])
    else:
        psP = pspool.tile([128, 256], F32, tag=f"pd{s}", name=f"psP{s}")
        _mm(nc, psP[:], ct["c_ayvit"], Fsb[:])   # DQy directly from Fsb
        S["psP"] = psP
    S["Qsb"] = Qsb


def _post(nc, wpool, pspool, ct, S, first):
    s, Bt, Ct, tx = S["s"], S["B"], S["Ct"], S["tx"]
    Qsb, psP, Clip = S["Qsb"], S["psP"], S["Clip"]
    Q3 = Qsb[:].rearrange("p (c j) -> p c j", j=64)
    t3 = tx[:].rearrange("p (c j) -> p c j", j=64)
    nc.gpsimd.tensor_tensor(t3[:, :, 0:63], Q3[:, :, 1:64], Q3[:, :, 0:63],
                            ALU.subtract)
    if first:
        nc.vector.tensor_copy(Ct[:, 0:256], tx[:])
        nc.vector.tensor_copy(Ct[:, 256:512], psP[:])
    else:
        nc.vector.tensor_tensor(Ct[:, 0:256], tx[:], Bt[:, 0:256], ALU.add)
        nc.vector.tensor_tensor(Ct[:, 256:512], psP[:], Bt[:, 256:512], ALU.add)

    nc.vector.tensor_scalar(Clip[:], Ct[:], -TH, TH, ALU.max, ALU.min)

    if first:
        nc.vector.tensor_scalar_mul(Bt[:], Clip[:], ETA)
        S["psb"] = None
    else:
        psb = pspool.tile([128, 512], F32, tag=f"pd{s}", name=f"psb{s}")
        _mm(nc, psb[:], ct["c_i09"], Bt[:], start=True, stop=False)
        _mm(nc, psb[:], ct["c_i01"], Clip[:], start=False, stop=True)
        S["psb"] = psb


_CACHE = {}


def _get_nc():
    if "nc" not in _CACHE:
        _CACHE["nc"] = build(reps=1)
    return _CACHE["nc"]


def kernel(F, image, D):
    """Full inputs in, full output out. `image` is unused (mask disabled)."""
    F = np.asarray(F, dtype=np.float32)
    D = np.asarray(D, dtype=np.float32)
    consts = host_constants(D)
    cblk = pack_consts(consts)
    per_core = host_pack(F)
    nc = _get_nc()
    in_maps = [{"x0": per_core[r], "consts": cblk} for r in range(NCORES)]
    res = run_bass_kernel_spmd(nc, in_maps, list(range(NCORES)))
    outs = [np.asarray(res.results[r]["out"]) for r in range(NCORES)]
    return host_unpack(outs)
